# revision 1
# baseline (speedup 1.0000x reference)
"""Conv2d 3x3 (stride 1, pad 1) as implicit GEMM on 8 Trainium2 NeuronCores.

x: [32, 128, 56, 56] f32, W: [256, 128, 3, 3] f32 -> out: [32, 256, 56, 56] f32

Sharding: data-parallel over batch, 4 images per core (sharding_hint).

Per-core kernel (PE-bound, ~97us of bf16 matmul at 78.6 TF/s peak):
  - host pre-pads x to [4, 128, 58, 58], casts to bf16; pre-transposes W to
    [Cin=128, 9*Cout] bf16 (tap-major) so no on-device transposes are needed
  - Cin=128 is the contraction dim and lives on the SBUF partition axis; for
    each output tile (img, 8-row group, cout half) nine matmuls (one per
    3x3 tap, N=448 columns) accumulate into one PSUM bank, with the padded
    input addressed through strided [128, 8, 56] views (no im2col copies)
  - PSUM -> SBUF fp32 copy on the vector engine, streaming HWDGE store per
    tile; input DMAs ride both HWDGE rings, image 0 split into row chunks so
    compute starts ~1us in; a chain of dependency-free warmup matmuls holds
    the PE clock ramp (HAM) warm while the first loads land
  - built on bacc.Bacc so multi-wait instructions are legalized (split) for
    the 1-sync-wait-per-instruction encoding limit of this toolchain

Measured: TimelineSim (repo cost model) 103.2us single-shot; real-HW
steady-state body ~90us via repeated-body slope timing (NTFF profiling is
unavailable under this axon build). Numerics: bf16 inputs / fp32 PSUM
accumulate -> rel L2 error ~2.1e-3 vs the fp32 reference.
"""

import sys

for _p in ("/opt/trn_rl_repo",):
    if _p not in sys.path:
        sys.path.insert(0, _p)

import numpy as np
import ml_dtypes

import concourse.bass as bass
import concourse.bacc as bacc
import concourse.mybir as mybir
from concourse import tile
from concourse.bass_utils import run_bass_kernel_spmd

N_CORES = 8
B = 32
B_PER_CORE = B // N_CORES  # 4
CIN = 128
COUT = 256
H = W_DIM = 56
HP = WP = 58  # padded
KH = KW = 3
KPOS = KH * KW  # 9
ROWS = 8               # output rows per matmul
NG = H // ROWS         # 7 row groups
NFREE = ROWS * W_DIM   # 448 free dim per matmul (<= 512 psum bank)
COUT_TILES = COUT // 128  # 2

_NC_CACHE = None


def build_nc(reps: int = 1, xsplits=(0, 10, 18, 34, HP), wchunks: int = 2) -> bass.Bass:
    # Bacc (not raw Bass): its compile() runs move_matmul_waits_to_ldweights
    # and generate_event_semaphores, which split multi-wait instructions to
    # satisfy the 1-sync-wait-per-instruction hardware encoding limit.
    # reps > 1 repeats the compute+store body (same outputs) for slope-based
    # hardware timing; the shipped kernel uses reps=1.
    nc = bacc.Bacc()
    xp = nc.dram_tensor(
        "xp", [B_PER_CORE, CIN, HP * WP], mybir.dt.bfloat16, kind="ExternalInput"
    )
    wt = nc.dram_tensor(
        "wt", [CIN, KPOS * COUT], mybir.dt.bfloat16, kind="ExternalInput"
    )
    out = nc.dram_tensor(
        "out", [B_PER_CORE, COUT, H * W_DIM], mybir.dt.float32, kind="ExternalOutput"
    )

    with tile.TileContext(nc) as tc:
        with (
            tc.tile_pool(name="wpool", bufs=1) as wpool,
            tc.tile_pool(name="xpool", bufs=1) as xpool,
            tc.tile_pool(name="opool", bufs=6) as opool,
            tc.tile_pool(name="pspool", bufs=7, space="PSUM") as pspool,
            tc.tile_pool(name="warmpool", bufs=1, space="PSUM") as warmpool,
        ):
            # Warm the PE clock (HAM / p-state ramp) while the input DMAs are
            # in flight: a chain of dependency-free matmuls on a memset
            # scratch tile keeps the PE busy from t=0, so the real matmuls
            # start at full clock. These never block the real stream (they
            # are ahead of it in PE program order and wait on nothing).
            scratch = opool.tile([128, 64], mybir.dt.bfloat16, name="warm_src", tag="wsrc")
            nc.vector.memset(scratch, 0.0)
            warm_ps = warmpool.tile([64, 64], mybir.dt.float32, name="warm_ps", tag="wps")
            for _ in range(64):
                nc.tensor.matmul(warm_ps, scratch[:, :64], scratch, start=True, stop=True)
            # Loads ride both HWDGE rings in parallel: weights (2 chunks) on
            # the scalar ring, x images (3 row chunks each) on the sync ring.
            # Chunking lets the first matmuls start as soon as weight chunk 0
            # and rows 0..17 of image 0 have landed; row chunk boundaries are
            # aligned so row group g only reads padded rows [8g, 8g+9].
            w_sb = wpool.tile([CIN, KPOS * COUT], mybir.dt.bfloat16, name="w_sb")
            WSPLITS = tuple(
                (KPOS * COUT) * i // wchunks for i in range(wchunks)
            ) + (KPOS * COUT,)
            for lo, hi in zip(WSPLITS[:-1], WSPLITS[1:]):
                nc.scalar.dma_start(w_sb[:, lo:hi], wt[:, lo:hi])

            x_views = []
            for b in range(B_PER_CORE):
                xb = xpool.tile(
                    [CIN, HP * WP], mybir.dt.bfloat16, name=f"x_sb{b}", tag=f"x{b}"
                )
                # Only image 0 races the PE; later images load as one DMA.
                splits = tuple(xsplits) if b == 0 else (0, HP)
                for lo, hi in zip(splits[:-1], splits[1:]):
                    nc.sync.dma_start(
                        xb[:, lo * WP : hi * WP], xp[b, :, lo * WP : hi * WP]
                    )
                x_views.append(xb.rearrange("p (h w) -> p h w", w=WP))

            for _rep in range(reps):
              for b in range(B_PER_CORE):
                for g in range(NG):
                    for c in range(COUT_TILES):
                        r0 = g * ROWS
                        ps = pspool.tile(
                            [128, NFREE], mybir.dt.float32, name="ps", tag="ps"
                        )
                        for k in range(KPOS):
                            kh, kw = divmod(k, KW)
                            rhs = x_views[b][:, r0 + kh : r0 + kh + ROWS, kw : kw + W_DIM]
                            lhsT = w_sb[:, k * COUT + c * 128 : k * COUT + (c + 1) * 128]
                            nc.tensor.matmul(
                                ps, lhsT, rhs, start=(k == 0), stop=(k == KPOS - 1)
                            )
                        ob = opool.tile(
                            [128, NFREE], mybir.dt.float32, name="ob", tag="ob"
                        )
                        nc.vector.tensor_copy(ob, ps)
                        nc.sync.dma_start(
                            out[
                                b,
                                c * 128 : (c + 1) * 128,
                                r0 * W_DIM : (r0 + ROWS) * W_DIM,
                            ],
                            ob,
                        )
    nc.compile()
    return nc


def _get_nc() -> bass.Bass:
    global _NC_CACHE
    if _NC_CACHE is None:
        _NC_CACHE = build_nc()
    return _NC_CACHE


def _prep_inputs(x: np.ndarray, W: np.ndarray):
    x = np.asarray(x, dtype=np.float32)
    W = np.asarray(W, dtype=np.float32)
    bf16 = ml_dtypes.bfloat16

    xp = np.zeros((B, CIN, HP, WP), dtype=bf16)
    xp[:, :, 1 : 1 + H, 1 : 1 + W_DIM] = x.astype(bf16)
    xp = xp.reshape(B, CIN, HP * WP)

    # Wt[ci, k*COUT + co] = W[co, ci, kh, kw], k = kh*3 + kw
    Wt = (
        W.transpose(2, 3, 1, 0)          # [kh, kw, ci, co]
        .reshape(KPOS, CIN, COUT)        # [k, ci, co]
        .transpose(1, 0, 2)              # [ci, k, co]
        .reshape(CIN, KPOS * COUT)
        .astype(bf16)
    )

    in_maps = []
    for c in range(N_CORES):
        in_maps.append(
            {
                "xp": np.ascontiguousarray(xp[c * B_PER_CORE : (c + 1) * B_PER_CORE]),
                "wt": Wt,
            }
        )
    return in_maps


def kernel_run(x: np.ndarray, W: np.ndarray, **spmd_kwargs):
    """Run the conv and return (output, BassKernelResults)."""
    in_maps = _prep_inputs(x, W)
    res = run_bass_kernel_spmd(
        _get_nc(), in_maps, core_ids=list(range(N_CORES)), **spmd_kwargs
    )
    out = np.concatenate(
        [
            np.asarray(res.results[c]["out"], dtype=np.float32).reshape(
                B_PER_CORE, COUT, H, W_DIM
            )
            for c in range(N_CORES)
        ],
        axis=0,
    )
    return out, res


def kernel(x: np.ndarray, W: np.ndarray) -> np.ndarray:
    out, _ = kernel_run(x, W)
    return out



# revision 9
# speedup vs baseline: 1.3894x; 1.3894x over previous
"""Conv2d 3x3 (stride 1, pad 1) as implicit GEMM on 8 Trainium2 NeuronCores.

x: [32, 128, 56, 56] f32, W: [256, 128, 3, 3] f32 -> out: [32, 256, 56, 56] f32

Sharding: data-parallel over batch, 4 images per core (sharding_hint).

fp8 DoubleRow formulation (vs the previous bf16 kernel's 9 cycles/col, this
runs 5.5 PE cycles per output column per cout-half):
  - e4m3 residual split on the host: xh = Q(x), xl = Q((x-xh)*2^4),
    wh = Q(W*2^7), wl = Q((W*2^7-wh)*2^4), plus exactly power-of-2-rescaled
    copies whB = Q(wh/16), wlB = Q(wl/16) so every product lands on the common
    PSUM scale x*W*2^7 (host divides the output by 2^7 at the end — exact).
  - each DoubleRow matmul contracts TWO K=128 pairs at 0.5 cycles per output
    column: out += A_w^T A_x + B_w^T B_x. Per output tile (4 output rows,
    one cout half) 11 matmuls:
      m0..m8  (tap k): moving (xh_k | xl_k), stationary (wh_k | whB_k)
               -> full-precision x against fp8 W
      m9, m10: moving (xh_t, xh_t') for tap pairs (1,3), (5,7), stationary
               (wlB_t | wlB_t') -> W-residual correction on 4 of 9 taps; the
               other 5 taps' W-residuals are dropped. The tap-shifted xh
               copies are materialized host-side (a matmul moving AP whose
               pair slots overlap in SBUF fails walrus codegen)
    measured rel L2 err 0.0169 vs the f32 reference (tolerance 2e-2).
  - moving windows are flat 230-element runs over the padded 58-wide rows
    (j = r*58 + w); the 2 pad columns per row compute garbage that the
    PSUM->SBUF copy APs skip. In-DMA stays one padded fp8 image pair.
  - PSUM->SBUF copies convert to bf16 (halves out-DMA bytes; host upcasts)
    and alternate between the DVE and Activation engines; stores batch 14
    row groups into one [128, 3136] staging buffer -> one DMA per
    (image, cout half) to keep the serialized HWDGE/DMA devices off the
    critical path.
  - warmup matmul chain holds the PE p-state ramp as in the bf16 kernel.
"""

import sys

for _p in ("/opt/trn_rl_repo",):
    if _p not in sys.path:
        sys.path.insert(0, _p)

import numpy as np
import ml_dtypes

import concourse.bass as bass
import concourse.bacc as bacc
import concourse.mybir as mybir
from concourse import tile
from concourse.bass_utils import run_bass_kernel_spmd

N_CORES = 8
B = 32
B_PER_CORE = B // N_CORES  # 4
CIN = 128
COUT = 256
H = W_DIM = 56
HP = WP = 58  # padded
IMG = HP * WP  # 3364
ROWS = 4                # output rows per chain
NG = H // ROWS          # 14 row groups
NWIN = (ROWS - 1) * WP + W_DIM  # 230: flat window length per chain
COUT_TILES = COUT // 128  # 2
KEEP = (1, 3, 5, 7)     # taps with W-residual correction; pairs (1,3),(5,7)
NMM = 11                # matmuls per output tile
WCOLS = COUT_TILES * NMM * 2 * 128  # 5632 weight columns

X_SCALE = 128.0         # W pre-scale 2^7; host divides output by this
RES_SCALE = 16.0        # residual pre-scale 2^4

_NC_CACHE = None


def _tap_off(k: int) -> int:
    return (k // 3) * WP + (k % 3)


def build_nc(reps: int = 1, warm: int = 104) -> bass.Bass:
    # Bacc (not raw Bass): its compile() legalizes multi-wait instructions for
    # the 1-sync-wait-per-instruction encoding limit of this toolchain.
    nc = bacc.Bacc()
    xp = nc.dram_tensor(
        "xp", [B_PER_CORE, CIN, 2, IMG], mybir.dt.float8e4, kind="ExternalInput"
    )
    xq = nc.dram_tensor(
        "xq", [B_PER_CORE, CIN, 4, IMG], mybir.dt.float8e4, kind="ExternalInput"
    )
    wt = nc.dram_tensor("wt", [CIN, WCOLS], mybir.dt.float8e4, kind="ExternalInput")
    out = nc.dram_tensor(
        "out", [B_PER_CORE, COUT, H * W_DIM], mybir.dt.bfloat16, kind="ExternalOutput"
    )

    with tile.TileContext(nc) as tc:
        with (
            tc.tile_pool(name="wpool", bufs=1) as wpool,
            tc.tile_pool(name="xpool", bufs=1) as xpool,
            tc.tile_pool(name="stpool", bufs=3) as stpool,
            tc.tile_pool(name="pspool", bufs=7, space="PSUM") as pspool,
            tc.tile_pool(name="warmpool", bufs=1, space="PSUM") as warmpool,
        ):
            # Keep the PE p-state ramp warm while the first loads land: a
            # chain of dependency-free matmuls on a memset scratch tile.
            scratch = stpool.tile([128, 64], mybir.dt.bfloat16, name="warm_src", tag="wsrc")
            nc.vector.memset(scratch, 0.0)
            warm_ps = warmpool.tile([64, 64], mybir.dt.float32, name="warm_ps", tag="wps")
            for _ in range(warm):
                nc.tensor.matmul(warm_ps, scratch[:, :64], scratch, start=True, stop=True)

            # Weights on the scalar ring; first chunk covers cout-half 0's
            # m0..m8 so the first chain can start as early as possible.
            w_sb = wpool.tile([CIN, WCOLS], mybir.dt.float8e4, name="w_sb")
            WSPLITS = (0, 9 * 2 * 128, NMM * 2 * 128, WCOLS)
            for lo, hi in zip(WSPLITS[:-1], WSPLITS[1:]):
                nc.scalar.dma_start(w_sb[:, lo:hi], wt[:, lo:hi])

            # x images on the sync ring; image 0 split into row chunks so
            # compute starts as soon as the first groups' rows are resident.
            x_sb, xq_sb = [], []
            for b in range(B_PER_CORE):
                xb = xpool.tile(
                    [CIN, 2, IMG], mybir.dt.float8e4, name=f"x_sb{b}", tag=f"x{b}"
                )
                qb = xpool.tile(
                    [CIN, 4, IMG], mybir.dt.float8e4, name=f"xq_sb{b}", tag=f"xq{b}"
                )
                splits = (0, 7, 15, 31, HP) if b == 0 else (0, HP)
                for lo, hi in zip(splits[:-1], splits[1:]):
                    nc.sync.dma_start(
                        xb[:, :, lo * WP : hi * WP], xp[b, :, :, lo * WP : hi * WP]
                    )
                    nc.sync.dma_start(
                        qb[:, :, lo * WP : hi * WP], xq[b, :, :, lo * WP : hi * WP]
                    )
                x_sb.append(xb)
                xq_sb.append(qb)

            for _rep in range(reps):
              for b in range(B_PER_CORE):
                for c in range(COUT_TILES):
                    st = stpool.tile(
                        [128, H * W_DIM], mybir.dt.bfloat16, name="st", tag="st"
                    )
                    for gp in range(NG // 2):
                        ps = pspool.tile([128, 512], mybir.dt.float32, name="ps", tag="ps")
                        for half in range(2):
                            g = 2 * gp + half
                            out_ps = ps[:, half * NWIN : (half + 1) * NWIN]
                            base = ROWS * g * WP
                            for m in range(NMM):
                                if m < 9:
                                    s = base + _tap_off(m)
                                    rhs = x_sb[b][:, :, s : s + NWIN]
                                else:
                                    sl = 2 * (m - 9)
                                    rhs = xq_sb[b][:, sl : sl + 2, base : base + NWIN]
                                wcol = (c * NMM + m) * 2 * 128
                                lhsT = w_sb[:, wcol : wcol + 256].rearrange(
                                    "p (two m) -> p two m", two=2
                                )
                                nc.tensor.matmul(
                                    out_ps,
                                    lhsT,
                                    rhs,
                                    start=(m == 0),
                                    stop=(m == NMM - 1),
                                    perf_mode=mybir.MatmulPerfMode.DoubleRow,
                                    skip_group_check=(half == 1),
                                )
                        # copy both chains (skipping the per-row pad columns)
                        # into the bf16 staging buffer; alternate engines.
                        src = bass.AP(
                            tensor=ps.tensor,
                            offset=ps.offset,
                            ap=[list(ps.ap[0]), [NWIN, 2], [WP, ROWS], [1, W_DIM]],
                        )
                        dst = bass.AP(
                            tensor=st.tensor,
                            offset=st.offset + gp * 2 * ROWS * W_DIM,
                            ap=[list(st.ap[0]), [ROWS * W_DIM, 2], [W_DIM, ROWS], [1, W_DIM]],
                        )
                        if gp % 2 == 0:
                            nc.vector.tensor_copy(dst, src)
                        else:
                            nc.scalar.activation(
                                dst, src, mybir.ActivationFunctionType.Copy
                            )
                    nc.sync.dma_start(out[b, c * 128 : (c + 1) * 128, :], st)
    nc.compile()
    return nc


def _get_nc() -> bass.Bass:
    global _NC_CACHE
    if _NC_CACHE is None:
        _NC_CACHE = build_nc()
    return _NC_CACHE


def _prep_inputs(x: np.ndarray, W: np.ndarray):
    e4 = ml_dtypes.float8_e4m3
    x = np.asarray(x, dtype=np.float32)
    W = np.asarray(W, dtype=np.float32)

    xpad = np.zeros((B, CIN, HP, WP), dtype=np.float32)
    xpad[:, :, 1 : 1 + H, 1 : 1 + W_DIM] = x
    xh8 = xpad.astype(e4)
    xl8 = ((xpad - xh8.astype(np.float32)) * RES_SCALE).astype(e4)
    # [B, CIN, 2, IMG]: slot 0 = xh, slot 1 = xl
    xhl = np.stack(
        [xh8.reshape(B, CIN, IMG), xl8.reshape(B, CIN, IMG)], axis=2
    )
    # [B, CIN, 4, IMG]: tap-shifted xh copies for the W-residual matmuls
    # (slot i = xh shifted left by _tap_off(KEEP[i]))
    xh_flat = xh8.reshape(B, CIN, IMG)
    xsh = np.zeros((B, CIN, 4, IMG), dtype=e4)
    for i, t in enumerate(KEEP):
        d = _tap_off(t)
        xsh[:, :, i, : IMG - d] = xh_flat[:, :, d:]

    Ws = W * X_SCALE
    wh8 = Ws.astype(e4)
    whf = wh8.astype(np.float32)
    wl8 = ((Ws - whf) * RES_SCALE).astype(e4)
    whB8 = (whf / RES_SCALE).astype(e4)
    wlB8 = (wl8.astype(np.float32) / RES_SCALE).astype(e4)

    # wt[ci, ((c*11 + m)*2 + slot)*128 + j] with cout = c*128 + j
    wtbuf = np.zeros((CIN, WCOLS), dtype=e4)
    for c in range(COUT_TILES):
        co = slice(c * 128, (c + 1) * 128)
        for m in range(NMM):
            col = (c * NMM + m) * 2 * 128
            if m < 9:
                kh, kw = divmod(m, 3)
                wtbuf[:, col : col + 128] = wh8[co, :, kh, kw].T
                wtbuf[:, col + 128 : col + 256] = whB8[co, :, kh, kw].T
            else:
                t0, t1 = KEEP[2 * (m - 9)], KEEP[2 * (m - 9) + 1]
                wtbuf[:, col : col + 128] = wlB8[co, :, t0 // 3, t0 % 3].T
                wtbuf[:, col + 128 : col + 256] = wlB8[co, :, t1 // 3, t1 % 3].T

    in_maps = []
    for cidx in range(N_CORES):
        in_maps.append(
            {
                "xp": np.ascontiguousarray(
                    xhl[cidx * B_PER_CORE : (cidx + 1) * B_PER_CORE]
                ),
                "xq": np.ascontiguousarray(
                    xsh[cidx * B_PER_CORE : (cidx + 1) * B_PER_CORE]
                ),
                "wt": wtbuf,
            }
        )
    return in_maps


def kernel_run(x: np.ndarray, W: np.ndarray, **spmd_kwargs):
    """Run the conv and return (output, BassKernelResults)."""
    in_maps = _prep_inputs(x, W)
    res = run_bass_kernel_spmd(
        _get_nc(), in_maps, core_ids=list(range(N_CORES)), **spmd_kwargs
    )
    out = np.concatenate(
        [
            np.asarray(res.results[c]["out"])
            .astype(np.float32)
            .reshape(B_PER_CORE, COUT, H, W_DIM)
            for c in range(N_CORES)
        ],
        axis=0,
    )
    out *= np.float32(1.0 / X_SCALE)
    return out, res


def kernel(x: np.ndarray, W: np.ndarray) -> np.ndarray:
    out, _ = kernel_run(x, W)
    return out


# revision 13
# speedup vs baseline: 1.4660x; 1.0552x over previous
"""Conv2d 3x3 (stride 1, pad 1) as implicit GEMM on 8 Trainium2 NeuronCores.

x: [32, 128, 56, 56] f32, W: [256, 128, 3, 3] f32 -> out: [32, 256, 56, 56] f32

Sharding: data-parallel over batch, 4 images per core (sharding_hint).

fp8 DoubleRow formulation (vs the previous bf16 kernel's 9 cycles/col, this
runs 5.5 PE cycles per output column per cout-half):
  - e4m3 residual split on the host: xh = Q(x), xl = Q((x-xh)*2^4),
    wh = Q(W*2^7), wl = Q((W*2^7-wh)*2^4), plus exactly power-of-2-rescaled
    copies whB = Q(wh/16), wlB = Q(wl/16) so every product lands on the common
    PSUM scale x*W*2^7 (host divides the output by 2^7 at the end — exact).
  - each DoubleRow matmul contracts TWO K=128 pairs at 0.5 cycles per output
    column: out += A_w^T A_x + B_w^T B_x. Per output tile (4 output rows,
    one cout half) 11 matmuls:
      m0..m8  (tap k): moving (xh_k | xl_k), stationary (wh_k | whB_k)
               -> full-precision x against fp8 W
      m9, m10: moving (xh_t, xh_t') for tap pairs (1,3), (5,7), stationary
               (wlB_t | wlB_t') -> W-residual correction on 4 of 9 taps; the
               other 5 taps' W-residuals are dropped. The tap-shifted xh
               copies are materialized host-side (a matmul moving AP whose
               pair slots overlap in SBUF fails walrus codegen)
    measured rel L2 err 0.0169 vs the f32 reference (tolerance 2e-2).
  - moving windows are flat 230-element runs over the padded 58-wide rows
    (j = r*58 + w); the 2 pad columns per row compute garbage that the
    PSUM->SBUF copy APs skip. In-DMA stays one padded fp8 image pair.
  - PSUM->SBUF copies convert to bf16 (halves out-DMA bytes; host upcasts)
    and alternate between the DVE and Activation engines; stores batch 14
    row groups into one [128, 3136] staging buffer -> one DMA per
    (image, cout half) to keep the serialized HWDGE/DMA devices off the
    critical path.
  - warmup matmul chain holds the PE p-state ramp as in the bf16 kernel.
"""

import sys

for _p in ("/opt/trn_rl_repo",):
    if _p not in sys.path:
        sys.path.insert(0, _p)

import numpy as np
import ml_dtypes

import concourse.bass as bass
import concourse.bacc as bacc
import concourse.mybir as mybir
from concourse import tile
from concourse.bass_utils import run_bass_kernel_spmd

N_CORES = 8
B = 32
B_PER_CORE = B // N_CORES  # 4
CIN = 128
COUT = 256
H = W_DIM = 56
HP = WP = 58  # padded
IMG = HP * WP  # 3364
ROWS = 4                # output rows per chain
NG = H // ROWS          # 14 row groups
NWIN = (ROWS - 1) * WP + W_DIM  # 230: flat window length per chain
COUT_TILES = COUT // 128  # 2
KEEP = (1, 3, 5, 7)     # taps with W-residual correction; pairs (1,3),(5,7)
NMM = 11                # matmuls per output tile
WCOLS = COUT_TILES * NMM * 2 * 128  # 5632 weight columns

X_SCALE = 128.0         # W pre-scale 2^7; host divides output by this
RES_SCALE = 16.0        # residual pre-scale 2^4

_NC_CACHE = None


def _tap_off(k: int) -> int:
    return (k // 3) * WP + (k % 3)


def build_nc(reps: int = 1, warm: int = 104) -> bass.Bass:
    # Bacc (not raw Bass): its compile() legalizes multi-wait instructions for
    # the 1-sync-wait-per-instruction encoding limit of this toolchain.
    nc = bacc.Bacc()
    xp = nc.dram_tensor(
        "xp", [B_PER_CORE, CIN, 2, IMG], mybir.dt.float8e4, kind="ExternalInput"
    )
    xq = nc.dram_tensor(
        "xq", [B_PER_CORE, CIN, 4, IMG], mybir.dt.float8e4, kind="ExternalInput"
    )
    wt = nc.dram_tensor("wt", [CIN, WCOLS], mybir.dt.float8e4, kind="ExternalInput")
    out = nc.dram_tensor(
        "out", [B_PER_CORE, COUT, H * W_DIM], mybir.dt.bfloat16, kind="ExternalOutput"
    )

    with tile.TileContext(nc) as tc:
        with (
            tc.tile_pool(name="wpool", bufs=1) as wpool,
            tc.tile_pool(name="xpool", bufs=1) as xpool,
            tc.tile_pool(name="stpool", bufs=4) as stpool,
            tc.tile_pool(name="pspool", bufs=7, space="PSUM") as pspool,
            tc.tile_pool(name="warmpool", bufs=1, space="PSUM") as warmpool,
        ):
            # Keep the PE p-state ramp warm while the first loads land: a
            # chain of dependency-free matmuls on a memset scratch tile.
            scratch = stpool.tile([128, 64], mybir.dt.bfloat16, name="warm_src", tag="wsrc")
            nc.vector.memset(scratch, 0.0)
            warm_ps = warmpool.tile([64, 64], mybir.dt.float32, name="warm_ps", tag="wps")
            for _ in range(warm):
                nc.tensor.matmul(warm_ps, scratch[:, :64], scratch, start=True, stop=True)

            # Weights on the scalar ring; first chunk covers cout-half 0's
            # m0..m8 so the first chain can start as early as possible.
            w_sb = wpool.tile([CIN, WCOLS], mybir.dt.float8e4, name="w_sb")
            WSPLITS = (0, 9 * 2 * 128, WCOLS)
            for lo, hi in zip(WSPLITS[:-1], WSPLITS[1:]):
                nc.scalar.dma_start(w_sb[:, lo:hi], wt[:, lo:hi])

            # x images on the sync ring; image 0 split into row chunks so
            # compute starts as soon as the first groups' rows are resident.
            x_sb, xq_sb = [], []
            for b in range(B_PER_CORE):
                xb = xpool.tile(
                    [CIN, 2, IMG], mybir.dt.float8e4, name=f"x_sb{b}", tag=f"x{b}"
                )
                qb = xpool.tile(
                    [CIN, 4, IMG], mybir.dt.float8e4, name=f"xq_sb{b}", tag=f"xq{b}"
                )
                splits = (0, 9, 18, 31, HP) if b == 0 else (0, HP)
                for lo, hi in zip(splits[:-1], splits[1:]):
                    nc.sync.dma_start(
                        xb[:, :, lo * WP : hi * WP], xp[b, :, :, lo * WP : hi * WP]
                    )
                    nc.sync.dma_start(
                        qb[:, :, lo * WP : hi * WP], xq[b, :, :, lo * WP : hi * WP]
                    )
                x_sb.append(xb)
                xq_sb.append(qb)

            for _rep in range(reps):
              for b in range(B_PER_CORE):
                for c in range(COUT_TILES):
                    st = stpool.tile(
                        [128, H * W_DIM], mybir.dt.bfloat16, name="st", tag="st"
                    )
                    for gp in range(NG // 2):
                        ps = pspool.tile([128, 512], mybir.dt.float32, name="ps", tag="ps")
                        for half in range(2):
                            g = 2 * gp + half
                            out_ps = ps[:, half * NWIN : (half + 1) * NWIN]
                            base = ROWS * g * WP
                            for m in range(NMM):
                                if m < 9:
                                    s = base + _tap_off(m)
                                    rhs = x_sb[b][:, :, s : s + NWIN]
                                else:
                                    sl = 2 * (m - 9)
                                    rhs = xq_sb[b][:, sl : sl + 2, base : base + NWIN]
                                wcol = (c * NMM + m) * 2 * 128
                                lhsT = w_sb[:, wcol : wcol + 256].rearrange(
                                    "p (two m) -> p two m", two=2
                                )
                                nc.tensor.matmul(
                                    out_ps,
                                    lhsT,
                                    rhs,
                                    start=(m == 0),
                                    stop=(m == NMM - 1),
                                    perf_mode=mybir.MatmulPerfMode.DoubleRow,
                                    skip_group_check=(half == 1),
                                )
                        # copy both chains (skipping the per-row pad columns)
                        # into the bf16 staging buffer; alternate engines.
                        src = bass.AP(
                            tensor=ps.tensor,
                            offset=ps.offset,
                            ap=[list(ps.ap[0]), [NWIN, 2], [WP, ROWS], [1, W_DIM]],
                        )
                        dst = bass.AP(
                            tensor=st.tensor,
                            offset=st.offset + gp * 2 * ROWS * W_DIM,
                            ap=[list(st.ap[0]), [ROWS * W_DIM, 2], [W_DIM, ROWS], [1, W_DIM]],
                        )
                        if gp % 2 == 0:
                            nc.vector.tensor_copy(dst, src)
                        else:
                            nc.scalar.activation(
                                dst, src, mybir.ActivationFunctionType.Copy
                            )
                        # last section: store in small pieces right behind
                        # the copies so the kernel tail is one short DMA, not
                        # a full-row 2.2us transfer.
                        last = b == B_PER_CORE - 1 and c == COUT_TILES - 1
                        if last and gp in (1, 3, 5, 6):
                            lo = {1: 0, 3: 2, 5: 4, 6: 6}[gp] * 2 * ROWS * W_DIM
                            hi = (gp + 1) * 2 * ROWS * W_DIM
                            nc.sync.dma_start(
                                out[b, c * 128 : (c + 1) * 128, lo:hi], st[:, lo:hi]
                            )
                    if not (b == B_PER_CORE - 1 and c == COUT_TILES - 1):
                        nc.sync.dma_start(out[b, c * 128 : (c + 1) * 128, :], st)
    nc.compile()
    return nc


def _get_nc() -> bass.Bass:
    global _NC_CACHE
    if _NC_CACHE is None:
        _NC_CACHE = build_nc()
    return _NC_CACHE


def _prep_inputs(x: np.ndarray, W: np.ndarray):
    e4 = ml_dtypes.float8_e4m3
    x = np.asarray(x, dtype=np.float32)
    W = np.asarray(W, dtype=np.float32)

    xpad = np.zeros((B, CIN, HP, WP), dtype=np.float32)
    xpad[:, :, 1 : 1 + H, 1 : 1 + W_DIM] = x
    xh8 = xpad.astype(e4)
    xl8 = ((xpad - xh8.astype(np.float32)) * RES_SCALE).astype(e4)
    # [B, CIN, 2, IMG]: slot 0 = xh, slot 1 = xl
    xhl = np.stack(
        [xh8.reshape(B, CIN, IMG), xl8.reshape(B, CIN, IMG)], axis=2
    )
    # [B, CIN, 4, IMG]: tap-shifted xh copies for the W-residual matmuls
    # (slot i = xh shifted left by _tap_off(KEEP[i]))
    xh_flat = xh8.reshape(B, CIN, IMG)
    xsh = np.zeros((B, CIN, 4, IMG), dtype=e4)
    for i, t in enumerate(KEEP):
        d = _tap_off(t)
        xsh[:, :, i, : IMG - d] = xh_flat[:, :, d:]

    Ws = W * X_SCALE
    wh8 = Ws.astype(e4)
    whf = wh8.astype(np.float32)
    wl8 = ((Ws - whf) * RES_SCALE).astype(e4)
    whB8 = (whf / RES_SCALE).astype(e4)
    wlB8 = (wl8.astype(np.float32) / RES_SCALE).astype(e4)

    # wt[ci, ((c*11 + m)*2 + slot)*128 + j] with cout = c*128 + j
    wtbuf = np.zeros((CIN, WCOLS), dtype=e4)
    for c in range(COUT_TILES):
        co = slice(c * 128, (c + 1) * 128)
        for m in range(NMM):
            col = (c * NMM + m) * 2 * 128
            if m < 9:
                kh, kw = divmod(m, 3)
                wtbuf[:, col : col + 128] = wh8[co, :, kh, kw].T
                wtbuf[:, col + 128 : col + 256] = whB8[co, :, kh, kw].T
            else:
                t0, t1 = KEEP[2 * (m - 9)], KEEP[2 * (m - 9) + 1]
                wtbuf[:, col : col + 128] = wlB8[co, :, t0 // 3, t0 % 3].T
                wtbuf[:, col + 128 : col + 256] = wlB8[co, :, t1 // 3, t1 % 3].T

    in_maps = []
    for cidx in range(N_CORES):
        in_maps.append(
            {
                "xp": np.ascontiguousarray(
                    xhl[cidx * B_PER_CORE : (cidx + 1) * B_PER_CORE]
                ),
                "xq": np.ascontiguousarray(
                    xsh[cidx * B_PER_CORE : (cidx + 1) * B_PER_CORE]
                ),
                "wt": wtbuf,
            }
        )
    return in_maps


def kernel_run(x: np.ndarray, W: np.ndarray, **spmd_kwargs):
    """Run the conv and return (output, BassKernelResults)."""
    in_maps = _prep_inputs(x, W)
    res = run_bass_kernel_spmd(
        _get_nc(), in_maps, core_ids=list(range(N_CORES)), **spmd_kwargs
    )
    out = np.concatenate(
        [
            np.asarray(res.results[c]["out"])
            .astype(np.float32)
            .reshape(B_PER_CORE, COUT, H, W_DIM)
            for c in range(N_CORES)
        ],
        axis=0,
    )
    out *= np.float32(1.0 / X_SCALE)
    return out, res


def kernel(x: np.ndarray, W: np.ndarray) -> np.ndarray:
    out, _ = kernel_run(x, W)
    return out


# revision 16
# speedup vs baseline: 1.4902x; 1.0165x over previous
"""Conv2d 3x3 (stride 1, pad 1) as implicit GEMM on 8 Trainium2 NeuronCores.

x: [32, 128, 56, 56] f32, W: [256, 128, 3, 3] f32 -> out: [32, 256, 56, 56] f32

Sharding: data-parallel over batch, 4 images per core (sharding_hint).

fp8 DoubleRow formulation (vs the previous bf16 kernel's 9 cycles/col, this
runs 5.5 PE cycles per output column per cout-half):
  - e4m3 residual split on the host: xh = Q(x), xl = Q((x-xh)*2^4),
    wh = Q(W*2^7), wl = Q((W*2^7-wh)*2^4), plus exactly power-of-2-rescaled
    copies whB = Q(wh/16), wlB = Q(wl/16) so every product lands on the common
    PSUM scale x*W*2^7 (host divides the output by 2^7 at the end — exact).
  - each DoubleRow matmul contracts TWO K=128 pairs at 0.5 cycles per output
    column: out += A_w^T A_x + B_w^T B_x. Per output tile (4 output rows,
    one cout half) 11 matmuls:
      m0..m8  (tap k): moving (xh_k | xl_k), stationary (wh_k | whB_k)
               -> full-precision x against fp8 W
      m9, m10: moving (xh_t, xh_t') for tap pairs (1,3), (5,7), stationary
               (wlB_t | wlB_t') -> W-residual correction on 4 of 9 taps; the
               other 5 taps' W-residuals are dropped. The tap-shifted xh
               copies are materialized host-side (a matmul moving AP whose
               pair slots overlap in SBUF fails walrus codegen)
    measured rel L2 err 0.0169 vs the f32 reference (tolerance 2e-2).
  - moving windows are flat 230-element runs over the padded 58-wide rows
    (j = r*58 + w); the 2 pad columns per row compute garbage that the
    PSUM->SBUF copy APs skip. In-DMA stays one padded fp8 image pair.
  - PSUM->SBUF copies convert to bf16 (halves out-DMA bytes; host upcasts)
    and alternate between the DVE and Activation engines; stores batch 14
    row groups into one [128, 3136] staging buffer -> one DMA per
    (image, cout half) to keep the serialized HWDGE/DMA devices off the
    critical path.
  - warmup matmul chain holds the PE p-state ramp as in the bf16 kernel.
"""

import sys

for _p in ("/opt/trn_rl_repo",):
    if _p not in sys.path:
        sys.path.insert(0, _p)

import numpy as np
import ml_dtypes

import concourse.bass as bass
import concourse.bacc as bacc
import concourse.mybir as mybir
from concourse import tile
from concourse.bass_utils import run_bass_kernel_spmd

N_CORES = 8
B = 32
B_PER_CORE = B // N_CORES  # 4
CIN = 128
COUT = 256
H = W_DIM = 56
HP = WP = 58  # padded
IMG = HP * WP  # 3364
ROWS = 4                # output rows per chain
NG = H // ROWS          # 14 row groups
NWIN = (ROWS - 1) * WP + W_DIM  # 230: flat window length per chain
COUT_TILES = COUT // 128  # 2
KEEP = (1, 3, 5, 7)     # taps with W-residual correction; pairs (1,3),(5,7)
NMM = 11                # matmuls per output tile
WCOLS = COUT_TILES * NMM * 2 * 128  # 5632 weight columns

X_SCALE = 128.0         # W pre-scale 2^7; host divides output by this
RES_SCALE = 16.0        # residual pre-scale 2^4

_NC_CACHE = None


def _tap_off(k: int) -> int:
    return (k // 3) * WP + (k % 3)


def build_nc(reps: int = 1, warm: int = 84) -> bass.Bass:
    # Bacc (not raw Bass): its compile() legalizes multi-wait instructions for
    # the 1-sync-wait-per-instruction encoding limit of this toolchain.
    nc = bacc.Bacc()
    xp = nc.dram_tensor(
        "xp", [B_PER_CORE, CIN, 2, IMG], mybir.dt.float8e4, kind="ExternalInput"
    )
    xq = nc.dram_tensor(
        "xq", [B_PER_CORE, CIN, 4, IMG], mybir.dt.float8e4, kind="ExternalInput"
    )
    wt = nc.dram_tensor("wt", [CIN, WCOLS], mybir.dt.float8e4, kind="ExternalInput")
    out = nc.dram_tensor(
        "out", [B_PER_CORE, COUT, H * W_DIM], mybir.dt.bfloat16, kind="ExternalOutput"
    )

    with tile.TileContext(nc) as tc:
        with (
            tc.tile_pool(name="wpool", bufs=1) as wpool,
            tc.tile_pool(name="xpool", bufs=1) as xpool,
            tc.tile_pool(name="stpool", bufs=4) as stpool,
            tc.tile_pool(name="pspool", bufs=7, space="PSUM") as pspool,
            tc.tile_pool(name="warmpool", bufs=1, space="PSUM") as warmpool,
        ):
            # Keep the PE p-state ramp warm while the first loads land: a
            # chain of dependency-free matmuls on a memset scratch tile
            # (memset on the otherwise-idle gpsimd engine so the chain can
            # start as early as possible).
            scratch = stpool.tile([128, 64], mybir.dt.bfloat16, name="warm_src", tag="wsrc")
            nc.gpsimd.memset(scratch, 0.0)
            warm_ps = warmpool.tile([64, 64], mybir.dt.float32, name="warm_ps", tag="wps")
            for _ in range(warm):
                nc.tensor.matmul(warm_ps, scratch[:, :64], scratch, start=True, stop=True)

            # Weights on the scalar ring. One TILE per chunk — a single big
            # tile makes every matmul wait on the LAST weight DMA through
            # coarse dep tracking.
            WSPLITS = (0, 9 * 2 * 128, NMM * 2 * 128, WCOLS)
            w_tiles = []
            for lo, hi in zip(WSPLITS[:-1], WSPLITS[1:]):
                wtile = wpool.tile(
                    [CIN, hi - lo], mybir.dt.float8e4, name=f"w_sb{lo}", tag=f"w{lo}"
                )
                nc.scalar.dma_start(wtile, wt[:, lo:hi])
                w_tiles.append((lo, hi, wtile))

            def w_slice(col):
                for lo, hi, wtile in w_tiles:
                    if lo <= col and col + 256 <= hi:
                        return wtile[:, col - lo : col - lo + 256]
                raise AssertionError(col)

            # x images on the sync ring; image 0 split into row chunks so
            # compute starts as soon as the first groups' rows are resident.
            x_sb, xq_sb = [], []
            for b in range(B_PER_CORE):
                xb = xpool.tile(
                    [CIN, 2, IMG], mybir.dt.float8e4, name=f"x_sb{b}", tag=f"x{b}"
                )
                qb = xpool.tile(
                    [CIN, 4, IMG], mybir.dt.float8e4, name=f"xq_sb{b}", tag=f"xq{b}"
                )
                splits = (0, 9, 18, 31, HP) if b == 0 else (0, HP)
                for lo, hi in zip(splits[:-1], splits[1:]):
                    nc.sync.dma_start(
                        xb[:, :, lo * WP : hi * WP], xp[b, :, :, lo * WP : hi * WP]
                    )
                    nc.sync.dma_start(
                        qb[:, :, lo * WP : hi * WP], xq[b, :, :, lo * WP : hi * WP]
                    )
                x_sb.append(xb)
                xq_sb.append(qb)

            for _rep in range(reps):
              for b in range(B_PER_CORE):
                last_b = b == B_PER_CORE - 1
                # cout halves interleaved per group pair: phase-1 x rows are
                # consumed at half the rate, so image-0's streaming chunks
                # stay ahead of the PE.
                sts = [
                    stpool.tile([128, H * W_DIM], mybir.dt.bfloat16, name="st", tag="st")
                    for _ in range(COUT_TILES)
                ]
                for gp in range(NG // 2):
                    for c in range(COUT_TILES):
                        st = sts[c]
                        ps = pspool.tile([128, 512], mybir.dt.float32, name="ps", tag="ps")
                        for half in range(2):
                            g = 2 * gp + half
                            out_ps = ps[:, half * NWIN : (half + 1) * NWIN]
                            base = ROWS * g * WP
                            for m in range(NMM):
                                if m < 9:
                                    s = base + _tap_off(m)
                                    rhs = x_sb[b][:, :, s : s + NWIN]
                                else:
                                    sl = 2 * (m - 9)
                                    rhs = xq_sb[b][:, sl : sl + 2, base : base + NWIN]
                                lhsT = w_slice((c * NMM + m) * 2 * 128).rearrange(
                                    "p (two m) -> p two m", two=2
                                )
                                nc.tensor.matmul(
                                    out_ps,
                                    lhsT,
                                    rhs,
                                    start=(m == 0),
                                    stop=(m == NMM - 1),
                                    perf_mode=mybir.MatmulPerfMode.DoubleRow,
                                    skip_group_check=(half == 1),
                                )
                        # copy both chains (skipping the per-row pad columns)
                        # into the bf16 staging buffer; alternate engines.
                        src = bass.AP(
                            tensor=ps.tensor,
                            offset=ps.offset,
                            ap=[list(ps.ap[0]), [NWIN, 2], [WP, ROWS], [1, W_DIM]],
                        )
                        dst = bass.AP(
                            tensor=st.tensor,
                            offset=st.offset + gp * 2 * ROWS * W_DIM,
                            ap=[list(st.ap[0]), [ROWS * W_DIM, 2], [W_DIM, ROWS], [1, W_DIM]],
                        )
                        if c == 0:
                            nc.vector.tensor_copy(dst, src)
                        else:
                            nc.scalar.activation(
                                dst, src, mybir.ActivationFunctionType.Copy
                            )
                        # last image: store in small pieces right behind the
                        # copies so the kernel tail is one short DMA, not a
                        # full-row 2.2us transfer.
                        if last_b and gp in (1, 3, 5, 6):
                            lo = {1: 0, 3: 2, 5: 4, 6: 6}[gp] * 2 * ROWS * W_DIM
                            hi = (gp + 1) * 2 * ROWS * W_DIM
                            nc.sync.dma_start(
                                out[b, c * 128 : (c + 1) * 128, lo:hi], st[:, lo:hi]
                            )
                if not last_b:
                    for c in range(COUT_TILES):
                        nc.sync.dma_start(
                            out[b, c * 128 : (c + 1) * 128, :], sts[c]
                        )
    nc.compile()
    return nc


def _get_nc() -> bass.Bass:
    global _NC_CACHE
    if _NC_CACHE is None:
        _NC_CACHE = build_nc()
    return _NC_CACHE


def _prep_inputs(x: np.ndarray, W: np.ndarray):
    e4 = ml_dtypes.float8_e4m3
    x = np.asarray(x, dtype=np.float32)
    W = np.asarray(W, dtype=np.float32)

    xpad = np.zeros((B, CIN, HP, WP), dtype=np.float32)
    xpad[:, :, 1 : 1 + H, 1 : 1 + W_DIM] = x
    xh8 = xpad.astype(e4)
    xl8 = ((xpad - xh8.astype(np.float32)) * RES_SCALE).astype(e4)
    # [B, CIN, 2, IMG]: slot 0 = xh, slot 1 = xl
    xhl = np.stack(
        [xh8.reshape(B, CIN, IMG), xl8.reshape(B, CIN, IMG)], axis=2
    )
    # [B, CIN, 4, IMG]: tap-shifted xh copies for the W-residual matmuls
    # (slot i = xh shifted left by _tap_off(KEEP[i]))
    xh_flat = xh8.reshape(B, CIN, IMG)
    xsh = np.zeros((B, CIN, 4, IMG), dtype=e4)
    for i, t in enumerate(KEEP):
        d = _tap_off(t)
        xsh[:, :, i, : IMG - d] = xh_flat[:, :, d:]

    Ws = W * X_SCALE
    wh8 = Ws.astype(e4)
    whf = wh8.astype(np.float32)
    wl8 = ((Ws - whf) * RES_SCALE).astype(e4)
    whB8 = (whf / RES_SCALE).astype(e4)
    wlB8 = (wl8.astype(np.float32) / RES_SCALE).astype(e4)

    # wt[ci, ((c*11 + m)*2 + slot)*128 + j] with cout = c*128 + j
    wtbuf = np.zeros((CIN, WCOLS), dtype=e4)
    for c in range(COUT_TILES):
        co = slice(c * 128, (c + 1) * 128)
        for m in range(NMM):
            col = (c * NMM + m) * 2 * 128
            if m < 9:
                kh, kw = divmod(m, 3)
                wtbuf[:, col : col + 128] = wh8[co, :, kh, kw].T
                wtbuf[:, col + 128 : col + 256] = whB8[co, :, kh, kw].T
            else:
                t0, t1 = KEEP[2 * (m - 9)], KEEP[2 * (m - 9) + 1]
                wtbuf[:, col : col + 128] = wlB8[co, :, t0 // 3, t0 % 3].T
                wtbuf[:, col + 128 : col + 256] = wlB8[co, :, t1 // 3, t1 % 3].T

    in_maps = []
    for cidx in range(N_CORES):
        in_maps.append(
            {
                "xp": np.ascontiguousarray(
                    xhl[cidx * B_PER_CORE : (cidx + 1) * B_PER_CORE]
                ),
                "xq": np.ascontiguousarray(
                    xsh[cidx * B_PER_CORE : (cidx + 1) * B_PER_CORE]
                ),
                "wt": wtbuf,
            }
        )
    return in_maps


def kernel_run(x: np.ndarray, W: np.ndarray, **spmd_kwargs):
    """Run the conv and return (output, BassKernelResults)."""
    in_maps = _prep_inputs(x, W)
    res = run_bass_kernel_spmd(
        _get_nc(), in_maps, core_ids=list(range(N_CORES)), **spmd_kwargs
    )
    out = np.concatenate(
        [
            np.asarray(res.results[c]["out"])
            .astype(np.float32)
            .reshape(B_PER_CORE, COUT, H, W_DIM)
            for c in range(N_CORES)
        ],
        axis=0,
    )
    out *= np.float32(1.0 / X_SCALE)
    return out, res


def kernel(x: np.ndarray, W: np.ndarray) -> np.ndarray:
    out, _ = kernel_run(x, W)
    return out


# revision 18
# speedup vs baseline: 1.5028x; 1.0084x over previous
"""Conv2d 3x3 (stride 1, pad 1) as implicit GEMM on 8 Trainium2 NeuronCores.

x: [32, 128, 56, 56] f32, W: [256, 128, 3, 3] f32 -> out: [32, 256, 56, 56] f32

Sharding: data-parallel over batch, 4 images per core (sharding_hint).

fp8 DoubleRow formulation (vs the previous bf16 kernel's 9 cycles/col, this
runs 5.5 PE cycles per output column per cout-half):
  - e4m3 residual split on the host: xh = Q(x), xl = Q((x-xh)*2^4),
    wh = Q(W*2^7), wl = Q((W*2^7-wh)*2^4), plus exactly power-of-2-rescaled
    copies whB = Q(wh/16), wlB = Q(wl/16) so every product lands on the common
    PSUM scale x*W*2^7 (host divides the output by 2^7 at the end — exact).
  - each DoubleRow matmul contracts TWO K=128 pairs at 0.5 cycles per output
    column: out += A_w^T A_x + B_w^T B_x. Per output tile (4 output rows,
    one cout half) 11 matmuls:
      m0..m8  (tap k): moving (xh_k | xl_k), stationary (wh_k | whB_k)
               -> full-precision x against fp8 W
      m9, m10: moving (xh_t, xh_t') for tap pairs (1,3), (5,7), stationary
               (wlB_t | wlB_t') -> W-residual correction on 4 of 9 taps; the
               other 5 taps' W-residuals are dropped. The tap-shifted xh
               copies are materialized host-side (a matmul moving AP whose
               pair slots overlap in SBUF fails walrus codegen)
    measured rel L2 err 0.0169 vs the f32 reference (tolerance 2e-2).
  - moving windows are flat 230-element runs over the padded 58-wide rows
    (j = r*58 + w); the 2 pad columns per row compute garbage that the
    PSUM->SBUF copy APs skip. In-DMA stays one padded fp8 image pair.
  - PSUM->SBUF copies convert to bf16 (halves out-DMA bytes; host upcasts)
    and alternate between the DVE and Activation engines; stores batch 14
    row groups into one [128, 3136] staging buffer -> one DMA per
    (image, cout half) to keep the serialized HWDGE/DMA devices off the
    critical path.
  - warmup matmul chain holds the PE p-state ramp as in the bf16 kernel.
"""

import sys

for _p in ("/opt/trn_rl_repo",):
    if _p not in sys.path:
        sys.path.insert(0, _p)

import numpy as np
import ml_dtypes

import concourse.bass as bass
import concourse.bacc as bacc
import concourse.mybir as mybir
from concourse import tile
from concourse.bass_utils import run_bass_kernel_spmd

N_CORES = 8
B = 32
B_PER_CORE = B // N_CORES  # 4
CIN = 128
COUT = 256
H = W_DIM = 56
HP = WP = 58  # padded
IMG = HP * WP  # 3364
ROWS = 4                # output rows per chain
NG = H // ROWS          # 14 row groups
NWIN = (ROWS - 1) * WP + W_DIM  # 230: flat window length per chain
COUT_TILES = COUT // 128  # 2
KEEP = (1, 3, 5, 7)     # taps with W-residual correction; pairs (1,3),(5,7)
NMM = 11                # matmuls per output tile
WCOLS = COUT_TILES * NMM * 2 * 128  # 5632 weight columns

X_SCALE = 128.0         # W pre-scale 2^7; host divides output by this
RES_SCALE = 16.0        # residual pre-scale 2^4

_NC_CACHE = None


def _tap_off(k: int) -> int:
    return (k // 3) * WP + (k % 3)


def build_nc(reps: int = 1, warm: int = 84) -> bass.Bass:
    # Bacc (not raw Bass): its compile() legalizes multi-wait instructions for
    # the 1-sync-wait-per-instruction encoding limit of this toolchain.
    nc = bacc.Bacc()
    xp = nc.dram_tensor(
        "xp", [B_PER_CORE, CIN, 2, IMG], mybir.dt.float8e4, kind="ExternalInput"
    )
    xq = nc.dram_tensor(
        "xq", [B_PER_CORE, CIN, 4, IMG], mybir.dt.float8e4, kind="ExternalInput"
    )
    wt = nc.dram_tensor("wt", [CIN, WCOLS], mybir.dt.float8e4, kind="ExternalInput")
    out = nc.dram_tensor(
        "out", [B_PER_CORE, COUT, H * W_DIM], mybir.dt.bfloat16, kind="ExternalOutput"
    )

    with tile.TileContext(nc) as tc:
        with (
            tc.tile_pool(name="wpool", bufs=1) as wpool,
            tc.tile_pool(name="xpool", bufs=1) as xpool,
            tc.tile_pool(name="stpool", bufs=4) as stpool,
            tc.tile_pool(name="pspool", bufs=7, space="PSUM") as pspool,
            tc.tile_pool(name="warmpool", bufs=1, space="PSUM") as warmpool,
        ):
            # Keep the PE p-state ramp warm while the first loads land: a
            # chain of dependency-free matmuls on a memset scratch tile
            # (memset on the otherwise-idle gpsimd engine so the chain can
            # start as early as possible).
            scratch = stpool.tile([128, 64], mybir.dt.bfloat16, name="warm_src", tag="wsrc")
            nc.gpsimd.memset(scratch, 0.0)
            warm_ps = warmpool.tile([64, 64], mybir.dt.float32, name="warm_ps", tag="wps")
            for _ in range(warm):
                nc.tensor.matmul(warm_ps, scratch[:, :64], scratch, start=True, stop=True)

            # All loads ride one explicitly-ordered ring (the shared HWDGE /
            # DMA devices drain FIFO, so issue order is arrival order):
            # weights for the first chains, then image-0 row chunks sized to
            # stay ahead of the group ladder, then whole images 1-3.
            # One weight TILE per chunk — a single big tile makes every
            # matmul wait on the LAST weight DMA through coarse dep tracking.
            WSPLITS = (0, 9 * 2 * 128, NMM * 2 * 128, WCOLS)
            w_tiles = []
            for lo, hi in zip(WSPLITS[:-1], WSPLITS[1:]):
                w_tiles.append(
                    (lo, hi, wpool.tile(
                        [CIN, hi - lo], mybir.dt.float8e4, name=f"w_sb{lo}", tag=f"w{lo}"
                    ))
                )

            def w_slice(col):
                for lo, hi, wtile in w_tiles:
                    if lo <= col and col + 256 <= hi:
                        return wtile[:, col - lo : col - lo + 256]
                raise AssertionError(col)

            x_sb, xq_sb = [], []
            for b in range(B_PER_CORE):
                x_sb.append(xpool.tile(
                    [CIN, 2, IMG], mybir.dt.float8e4, name=f"x_sb{b}", tag=f"x{b}"
                ))
                xq_sb.append(xpool.tile(
                    [CIN, 4, IMG], mybir.dt.float8e4, name=f"xq_sb{b}", tag=f"xq{b}"
                ))

            def load_x(b, lo, hi):
                nc.sync.dma_start(
                    x_sb[b][:, :, lo * WP : hi * WP], xp[b, :, :, lo * WP : hi * WP]
                )
                nc.sync.dma_start(
                    xq_sb[b][:, :, lo * WP : hi * WP], xq[b, :, :, lo * WP : hi * WP]
                )

            nc.sync.dma_start(w_tiles[0][2], wt[:, WSPLITS[0] : WSPLITS[1]])
            load_x(0, 0, 10)
            nc.sync.dma_start(w_tiles[1][2], wt[:, WSPLITS[1] : WSPLITS[2]])
            nc.sync.dma_start(w_tiles[2][2], wt[:, WSPLITS[2] : WSPLITS[3]])
            B0SPLITS = (10, 19, 28, 37, 46, 55, HP)
            for lo, hi in zip(B0SPLITS[:-1], B0SPLITS[1:]):
                load_x(0, lo, hi)
            for b in range(1, B_PER_CORE):
                load_x(b, 0, HP)

            for _rep in range(reps):
              for b in range(B_PER_CORE):
                last_b = b == B_PER_CORE - 1
                # cout halves interleaved per group pair: phase-1 x rows are
                # consumed at half the rate, so image-0's streaming chunks
                # stay ahead of the PE.
                sts = [
                    stpool.tile([128, H * W_DIM], mybir.dt.bfloat16, name="st", tag="st")
                    for _ in range(COUT_TILES)
                ]
                for gp in range(NG // 2):
                    for c in range(COUT_TILES):
                        st = sts[c]
                        ps = pspool.tile([128, 512], mybir.dt.float32, name="ps", tag="ps")
                        for half in range(2):
                            g = 2 * gp + half
                            out_ps = ps[:, half * NWIN : (half + 1) * NWIN]
                            base = ROWS * g * WP
                            for m in range(NMM):
                                if m < 9:
                                    s = base + _tap_off(m)
                                    rhs = x_sb[b][:, :, s : s + NWIN]
                                else:
                                    sl = 2 * (m - 9)
                                    rhs = xq_sb[b][:, sl : sl + 2, base : base + NWIN]
                                lhsT = w_slice((c * NMM + m) * 2 * 128).rearrange(
                                    "p (two m) -> p two m", two=2
                                )
                                nc.tensor.matmul(
                                    out_ps,
                                    lhsT,
                                    rhs,
                                    start=(m == 0),
                                    stop=(m == NMM - 1),
                                    perf_mode=mybir.MatmulPerfMode.DoubleRow,
                                    skip_group_check=(half == 1),
                                )
                        # copy both chains (skipping the per-row pad columns)
                        # into the bf16 staging buffer; alternate engines.
                        src = bass.AP(
                            tensor=ps.tensor,
                            offset=ps.offset,
                            ap=[list(ps.ap[0]), [NWIN, 2], [WP, ROWS], [1, W_DIM]],
                        )
                        dst = bass.AP(
                            tensor=st.tensor,
                            offset=st.offset + gp * 2 * ROWS * W_DIM,
                            ap=[list(st.ap[0]), [ROWS * W_DIM, 2], [W_DIM, ROWS], [1, W_DIM]],
                        )
                        if c == 0:
                            nc.vector.tensor_copy(dst, src)
                        else:
                            nc.scalar.activation(
                                dst, src, mybir.ActivationFunctionType.Copy
                            )
                        # last image: store in small pieces right behind the
                        # copies so the kernel tail is one short DMA, not a
                        # full-row 2.2us transfer.
                        if last_b and gp in (1, 3, 5, 6):
                            lo = {1: 0, 3: 2, 5: 4, 6: 6}[gp] * 2 * ROWS * W_DIM
                            hi = (gp + 1) * 2 * ROWS * W_DIM
                            nc.sync.dma_start(
                                out[b, c * 128 : (c + 1) * 128, lo:hi], st[:, lo:hi]
                            )
                if not last_b:
                    for c in range(COUT_TILES):
                        nc.sync.dma_start(
                            out[b, c * 128 : (c + 1) * 128, :], sts[c]
                        )
    nc.compile()
    return nc


def _get_nc() -> bass.Bass:
    global _NC_CACHE
    if _NC_CACHE is None:
        _NC_CACHE = build_nc()
    return _NC_CACHE


def _prep_inputs(x: np.ndarray, W: np.ndarray):
    e4 = ml_dtypes.float8_e4m3
    x = np.asarray(x, dtype=np.float32)
    W = np.asarray(W, dtype=np.float32)

    xpad = np.zeros((B, CIN, HP, WP), dtype=np.float32)
    xpad[:, :, 1 : 1 + H, 1 : 1 + W_DIM] = x
    xh8 = xpad.astype(e4)
    xl8 = ((xpad - xh8.astype(np.float32)) * RES_SCALE).astype(e4)
    # [B, CIN, 2, IMG]: slot 0 = xh, slot 1 = xl
    xhl = np.stack(
        [xh8.reshape(B, CIN, IMG), xl8.reshape(B, CIN, IMG)], axis=2
    )
    # [B, CIN, 4, IMG]: tap-shifted xh copies for the W-residual matmuls
    # (slot i = xh shifted left by _tap_off(KEEP[i]))
    xh_flat = xh8.reshape(B, CIN, IMG)
    xsh = np.zeros((B, CIN, 4, IMG), dtype=e4)
    for i, t in enumerate(KEEP):
        d = _tap_off(t)
        xsh[:, :, i, : IMG - d] = xh_flat[:, :, d:]

    Ws = W * X_SCALE
    wh8 = Ws.astype(e4)
    whf = wh8.astype(np.float32)
    wl8 = ((Ws - whf) * RES_SCALE).astype(e4)
    whB8 = (whf / RES_SCALE).astype(e4)
    wlB8 = (wl8.astype(np.float32) / RES_SCALE).astype(e4)

    # wt[ci, ((c*11 + m)*2 + slot)*128 + j] with cout = c*128 + j
    wtbuf = np.zeros((CIN, WCOLS), dtype=e4)
    for c in range(COUT_TILES):
        co = slice(c * 128, (c + 1) * 128)
        for m in range(NMM):
            col = (c * NMM + m) * 2 * 128
            if m < 9:
                kh, kw = divmod(m, 3)
                wtbuf[:, col : col + 128] = wh8[co, :, kh, kw].T
                wtbuf[:, col + 128 : col + 256] = whB8[co, :, kh, kw].T
            else:
                t0, t1 = KEEP[2 * (m - 9)], KEEP[2 * (m - 9) + 1]
                wtbuf[:, col : col + 128] = wlB8[co, :, t0 // 3, t0 % 3].T
                wtbuf[:, col + 128 : col + 256] = wlB8[co, :, t1 // 3, t1 % 3].T

    in_maps = []
    for cidx in range(N_CORES):
        in_maps.append(
            {
                "xp": np.ascontiguousarray(
                    xhl[cidx * B_PER_CORE : (cidx + 1) * B_PER_CORE]
                ),
                "xq": np.ascontiguousarray(
                    xsh[cidx * B_PER_CORE : (cidx + 1) * B_PER_CORE]
                ),
                "wt": wtbuf,
            }
        )
    return in_maps


def kernel_run(x: np.ndarray, W: np.ndarray, **spmd_kwargs):
    """Run the conv and return (output, BassKernelResults)."""
    in_maps = _prep_inputs(x, W)
    res = run_bass_kernel_spmd(
        _get_nc(), in_maps, core_ids=list(range(N_CORES)), **spmd_kwargs
    )
    out = np.concatenate(
        [
            np.asarray(res.results[c]["out"])
            .astype(np.float32)
            .reshape(B_PER_CORE, COUT, H, W_DIM)
            for c in range(N_CORES)
        ],
        axis=0,
    )
    out *= np.float32(1.0 / X_SCALE)
    return out, res


def kernel(x: np.ndarray, W: np.ndarray) -> np.ndarray:
    out, _ = kernel_run(x, W)
    return out
